# revision 12
# baseline (speedup 1.0000x reference)
"""nn_DenseGrid trilinear embedding lookup on 8 Trainium2 cores.

Strategy (data-parallel over points, codebook replicated per core):
  - 2,097,152 points sharded 8 ways (262,144 per core); full output gathered
    on host by concatenation.
  - Per core, points are processed in super-chunks of 128*F (partition p,
    slot f). For each point: fold transform+scale into q = A@p + b, floor
    (magic-number round + fixup, no reliance on HW cast rounding mode),
    fractional weights, base row index = x + 128y + 16384z.
  - A y-pair table P2[j] = [cb[j], cb[j+128]] (2x codebook) is built once
    per core with large contiguous DMAs + on-chip DVE interleave. One 288B
    gather descriptor starting at entry j then covers all 4 xy-corners of
    cell j, so each point needs only 2 descriptors (z0, z1).
  - Gather: indirect DMA, one descriptor per partition per instruction (the
    only mode trn2 walrus supports; ~1.45us per 128 descriptors, so
    instruction count dominates the runtime).
  - Interpolation: G *= W8 (8 corner weights broadcast over 18 features),
    then in-place tree reduction 144 -> 72 -> 36 -> 18 per point; strided
    store back to DRAM.
"""

import numpy as np

RES = 128
FEAT = 18
V = RES**3
MAGIC = float(2**23)
P = 128
N_CORES = 8
F = 64                      # point slots per partition per super-chunk

_cache = {}


def _build(n_points, A, b):
    import os
    os.environ.setdefault("NEURON_SCRATCHPAD_PAGE_SIZE", "320")
    import concourse.bass as bass
    import concourse.bacc as bacc
    import concourse.mybir as mybir
    import concourse.tile as tile

    f32 = mybir.dt.float32
    i32 = mybir.dt.int32
    Copy = mybir.ActivationFunctionType.Copy
    Op = mybir.AluOpType

    chunk = P * F
    n_chunks = n_points // chunk
    assert n_chunks * chunk == n_points

    nc = bacc.Bacc(None, target_bir_lowering=False, debug=False)
    pts = nc.declare_dram_parameter("pts", [n_points, 3], f32, isOutput=False)
    cb = nc.declare_dram_parameter("codebook", [V, FEAT], f32, isOutput=False)
    out = nc.declare_dram_parameter("out", [n_points, FEAT], f32, isOutput=True)

    # y-pair table: P2[j] = [cb[j], cb[j+128]] (36 floats). One 72-float
    # descriptor starting at entry j covers entries j, j+1 = the 4 xy-corners
    # (x0y0, x0y1, x1y0, x1y1) of cell base row j.
    p2 = nc.dram_tensor("p2tab", [V, 2 * FEAT], f32)
    with tile.TileContext(nc) as tc:
        with (
            tc.tile_pool(name="build", bufs=2) as bpool,
            tc.tile_pool(name="g", bufs=2) as gpool,
            tc.tile_pool(name="small", bufs=2) as spool,
        ):
            # Interleave on-chip: load rows [r0, r0+R) and [r0+128, r0+128+R)
            # into two tiles (pairs line up per partition), DVE-interleave into
            # [row, pair, 18], store contiguous. All DMAs are big & contiguous.
            ROWS = 8192
            RPP = ROWS // P
            n_bchunks = V // ROWS
            for ci in range(n_bchunks):
                r0 = ci * ROWS
                At = bpool.tile([P, RPP * FEAT], f32, tag="BA")
                Bt = bpool.tile([P, RPP * FEAT], f32, tag="BB")
                nc.scalar.dma_start(
                    out=At[:],
                    in_=cb[r0 : r0 + ROWS, :].rearrange("(p r) e -> p (r e)", p=P))
                if ci < n_bchunks - 1:
                    nc.scalar.dma_start(
                        out=Bt[:],
                        in_=cb[r0 + P : r0 + P + ROWS, :].rearrange("(p r) e -> p (r e)", p=P))
                else:
                    # last 128 pair rows are out of range (y=127 entries,
                    # never indexed) -> zero-fill
                    nc.vector.memset(Bt[:], 0.0)
                    nc.scalar.dma_start(
                        out=Bt[:126, :],
                        in_=cb[r0 + P : V, :].rearrange("(p r) e -> p (r e)", p=126))
                Ot = bpool.tile([P, RPP, 2, FEAT], f32, tag="BO")
                nc.vector.tensor_copy(out=Ot[:, :, 0, :],
                                      in_=At[:].rearrange("p (r e) -> p r e", e=FEAT))
                nc.vector.tensor_copy(out=Ot[:, :, 1, :],
                                      in_=Bt[:].rearrange("p (r e) -> p r e", e=FEAT))
                nc.sync.dma_start(
                    out=p2[r0 : r0 + ROWS, :].rearrange("(p r) e -> p (r e)", p=P),
                    in_=Ot[:].rearrange("p r t e -> p (r t e)"))
            k_hyb = min(4, n_chunks // 2)
            for c in range(n_chunks):
                c0 = c * chunk
                PT = spool.tile([P, 3 * F], f32, tag="PT")
                nc.sync.dma_start(
                    out=PT[:],
                    in_=pts[c0 : c0 + chunk, :].rearrange("(p f) c -> p (f c)", p=P),
                )
                PT3 = PT[:].rearrange("p (f c) -> p f c", c=3)

                Q = spool.tile([P, 3, F], f32, tag="Q")
                FL = spool.tile([P, 3, F], f32, tag="FL")
                W = spool.tile([P, 3, F], f32, tag="W")
                U = spool.tile([P, 3, F], f32, tag="U")
                T = spool.tile([P, 3, F], f32, tag="T")
                # q_k = A[k,0]x + A[k,1]y + A[k,2]z + b_k
                for k in range(3):
                    nc.scalar.activation(Q[:, k, :], PT3[:, :, 0], Copy,
                                         bias=float(b[k]), scale=float(A[k][0]))
                    nc.scalar.activation(T[:, k, :], PT3[:, :, 1], Copy,
                                         bias=0.0, scale=float(A[k][1]))
                    nc.vector.tensor_tensor(out=Q[:, k, :], in0=Q[:, k, :], in1=T[:, k, :], op=Op.add)
                    nc.scalar.activation(T[:, k, :], PT3[:, :, 2], Copy,
                                         bias=0.0, scale=float(A[k][2]))
                    nc.vector.tensor_tensor(out=Q[:, k, :], in0=Q[:, k, :], in1=T[:, k, :], op=Op.add)
                # floor(q): round-to-nearest via magic constant, then fix up
                nc.scalar.activation(T[:], Q[:], Copy, bias=MAGIC)
                nc.scalar.activation(FL[:], T[:], Copy, bias=-MAGIC)
                nc.vector.tensor_tensor(out=T[:], in0=FL[:], in1=Q[:], op=Op.is_gt)
                nc.vector.tensor_tensor(out=FL[:], in0=FL[:], in1=T[:], op=Op.subtract)
                # frac weights (from unclipped floor), then clip floor to [0,126]
                nc.vector.tensor_tensor(out=W[:], in0=Q[:], in1=FL[:], op=Op.subtract)
                nc.vector.tensor_scalar(out=FL[:], in0=FL[:], scalar1=0.0, scalar2=float(RES - 2),
                                        op0=Op.max, op1=Op.min)
                nc.scalar.activation(U[:], W[:], Copy, bias=1.0, scale=-1.0)

                hyb = c < k_hyb
                W4 = spool.tile([P, 4, F], f32, tag="W4")
                W8 = spool.tile([P, F, 8], f32, tag="W8")
                if hyb:
                    # (y,z) plane weights; W8 slot = 2*plane + dx
                    nc.vector.tensor_tensor(out=W4[:, 0, :], in0=U[:, 1, :], in1=U[:, 2, :], op=Op.mult)
                    nc.vector.tensor_tensor(out=W4[:, 1, :], in0=W[:, 1, :], in1=U[:, 2, :], op=Op.mult)
                    nc.vector.tensor_tensor(out=W4[:, 2, :], in0=U[:, 1, :], in1=W[:, 2, :], op=Op.mult)
                    nc.vector.tensor_tensor(out=W4[:, 3, :], in0=W[:, 1, :], in1=W[:, 2, :], op=Op.mult)
                    for k in range(4):
                        nc.vector.tensor_tensor(out=W8[:, :, 2 * k], in0=W4[:, k, :], in1=U[:, 0, :], op=Op.mult)
                        nc.vector.tensor_tensor(out=W8[:, :, 2 * k + 1], in0=W4[:, k, :], in1=W[:, 0, :], op=Op.mult)
                else:
                    # xy corner weights (dx major, dy minor), then scale by z
                    nc.vector.tensor_tensor(out=W4[:, 0, :], in0=U[:, 0, :], in1=U[:, 1, :], op=Op.mult)
                    nc.vector.tensor_tensor(out=W4[:, 1, :], in0=U[:, 0, :], in1=W[:, 1, :], op=Op.mult)
                    nc.vector.tensor_tensor(out=W4[:, 2, :], in0=W[:, 0, :], in1=U[:, 1, :], op=Op.mult)
                    nc.vector.tensor_tensor(out=W4[:, 3, :], in0=W[:, 0, :], in1=W[:, 1, :], op=Op.mult)
                    for k in range(4):
                        nc.vector.tensor_tensor(out=W8[:, :, k], in0=W4[:, k, :], in1=U[:, 2, :], op=Op.mult)
                        nc.vector.tensor_tensor(out=W8[:, :, 4 + k], in0=W4[:, k, :], in1=W[:, 2, :], op=Op.mult)

                # base row index = fx + 128 fy + 16384 fz  (exact in f32)
                B = spool.tile([P, F], f32, tag="B")
                T2 = spool.tile([P, 2, F], f32, tag="T2")
                nc.scalar.activation(T2[:, 0, :], FL[:, 1, :], Copy, scale=float(RES))
                nc.scalar.activation(T2[:, 1, :], FL[:, 2, :], Copy, scale=float(RES * RES))
                nc.vector.tensor_tensor(out=B[:], in0=FL[:, 0, :], in1=T2[:, 0, :], op=Op.add)
                nc.vector.tensor_tensor(out=B[:], in0=B[:], in1=T2[:, 1, :], op=Op.add)
                G = gpool.tile([P, F, 2, 72], f32, tag="G")
                if hyb:
                    # gather straight from cb (4 x-pair descs/point) while the
                    # p2 build is still streaming on the DMA queues
                    IDX4 = spool.tile([P, F, 4], i32, tag="IDX4")
                    nc.vector.tensor_copy(out=IDX4[:, :, 0], in_=B[:])
                    for k, off in ((1, RES), (2, RES * RES), (3, RES * RES + RES)):
                        nc.vector.tensor_scalar(out=IDX4[:, :, k], in0=B[:],
                                                scalar1=float(off), scalar2=None, op0=Op.add)
                    G4 = G[:].rearrange("p f z e -> p (f z e)").rearrange(
                        "p (f c e) -> p f c e", c=4, e=36)
                    for g in range(F):
                        for cc in range(4):
                            nc.gpsimd.indirect_dma_start(
                                out=G4[:, g, cc, :],
                                out_offset=None,
                                in_=cb[:],
                                in_offset=bass.IndirectOffsetOnAxis(ap=IDX4[:, g, cc : cc + 1], axis=0),
                            )
                else:
                    IDX = spool.tile([P, F, 2], i32, tag="IDX")
                    nc.vector.tensor_copy(out=IDX[:, :, 0], in_=B[:])
                    nc.vector.tensor_scalar(out=IDX[:, :, 1], in0=B[:], scalar1=float(RES * RES),
                                            scalar2=None, op0=Op.add)
                    # per point-slot f, per z-plane: 72 floats = 4 xy corners
                    for g in range(F):
                        for zz in range(2):
                            nc.gpsimd.indirect_dma_start(
                                out=G[:, g, zz, :],
                                out_offset=None,
                                in_=p2[:],
                                in_offset=bass.IndirectOffsetOnAxis(ap=IDX[:, g, zz : zz + 1], axis=0),
                            )

                # weighted multiply + in-place tree reduction
                Gv = G[:].rearrange("p f z e -> p (f z e)").rearrange(
                    "p (f d j) -> p f d j", d=8, j=FEAT)
                W8b = W8[:].unsqueeze(-1).broadcast_to([P, F, 8, FEAT])
                nc.vector.tensor_tensor(out=Gv, in0=Gv, in1=W8b, op=Op.mult)
                Gf = G[:].rearrange("p f z e -> p (f z e)")
                for width in (72, 36, 18):
                    a = Gf.rearrange("p (f e) -> p f e", e=144)[:, :, 0:width]
                    bb = Gf.rearrange("p (f e) -> p f e", e=144)[:, :, width : 2 * width]
                    nc.vector.tensor_tensor(out=a, in0=a, in1=bb, op=Op.add)

                res = Gf.rearrange("p (f e) -> p f e", e=144)[:, :, 0:FEAT]
                nc.sync.dma_start(
                    out=out[c0 : c0 + chunk, :].rearrange("(p f) c -> p (f c)", p=P),
                    in_=res,
                )
    nc.finalize()
    return nc


def kernel(pts, codebook, transform, _trace=False):
    from concourse.bass_utils import run_bass_kernel_spmd

    pts = np.asarray(pts, dtype=np.float32)
    codebook = np.ascontiguousarray(np.asarray(codebook, dtype=np.float32))
    transform = np.asarray(transform, dtype=np.float32)

    p_flat = np.ascontiguousarray(pts.reshape(-1, 3))
    n_total = p_flat.shape[0]
    n_per = n_total // N_CORES
    assert n_per * N_CORES == n_total

    # fold transform inverse + grid scale into affine q = A p + b (host side,
    # 4x4 input only)
    R_inv = np.linalg.inv(transform[:3, :3].astype(np.float64))
    A = (RES - 1) * R_inv
    b = -A @ transform[:3, 3].astype(np.float64)

    key = (n_per, A.tobytes(), b.tobytes())
    if key not in _cache:
        _cache[key] = _build(n_per, A, b)
    nc = _cache[key]

    in_maps = [
        {"pts": p_flat[i * n_per : (i + 1) * n_per], "codebook": codebook}
        for i in range(N_CORES)
    ]
    r = run_bass_kernel_spmd(nc, in_maps, list(range(N_CORES)), trace=_trace)
    kernel.last_exec_time_ns = r.exec_time_ns
    out = np.concatenate([r.results[i]["out"] for i in range(N_CORES)], axis=0)
    return out


kernel.last_exec_time_ns = None


# revision 13
# speedup vs baseline: 1.0506x; 1.0506x over previous
"""nn_DenseGrid trilinear embedding lookup on 8 Trainium2 cores.

Strategy (data-parallel over points, codebook replicated per core):
  - 2,097,152 points sharded 8 ways (262,144 per core); full output gathered
    on host by concatenation.
  - Per core, points are processed in super-chunks of 128*F (partition p,
    slot f). For each point: fold transform+scale into q = A@p + b, floor
    (magic-number round + fixup, no reliance on HW cast rounding mode),
    fractional weights, base row index = x + 128y + 16384z.
  - A y-pair table P2[j] = [cb[j], cb[j+128]] (2x codebook) is built once
    per core with large contiguous DMAs + on-chip DVE interleave. One 288B
    gather descriptor starting at entry j then covers all 4 xy-corners of
    cell j, so each point needs only 2 descriptors (z0, z1).
  - Gather: indirect DMA, one descriptor per partition per instruction (the
    only mode trn2 walrus supports; ~1.45us per 128 descriptors, so
    instruction count dominates the runtime).
  - Interpolation: G *= W8 (8 corner weights broadcast over 18 features),
    then in-place tree reduction 144 -> 72 -> 36 -> 18 per point; strided
    store back to DRAM.
"""

import numpy as np

RES = 128
FEAT = 18
V = RES**3
MAGIC = float(2**23)
P = 128
N_CORES = 8
F = 64                      # point slots per partition per super-chunk

_cache = {}


def _build(n_points, A, b):
    import os
    os.environ.setdefault("NEURON_SCRATCHPAD_PAGE_SIZE", "320")
    import concourse.bass as bass
    import concourse.bacc as bacc
    import concourse.mybir as mybir
    import concourse.tile as tile

    f32 = mybir.dt.float32
    i32 = mybir.dt.int32
    Copy = mybir.ActivationFunctionType.Copy
    Op = mybir.AluOpType

    chunk = P * F
    n_chunks = n_points // chunk
    assert n_chunks * chunk == n_points

    nc = bacc.Bacc(None, target_bir_lowering=False, debug=False)
    pts = nc.declare_dram_parameter("pts", [n_points, 3], f32, isOutput=False)
    cb = nc.declare_dram_parameter("codebook", [V, FEAT], f32, isOutput=False)
    out = nc.declare_dram_parameter("out", [n_points, FEAT], f32, isOutput=True)

    # y-pair table: P2[j] = [cb[j], cb[j+128]] (36 floats). One 72-float
    # descriptor starting at entry j covers entries j, j+1 = the 4 xy-corners
    # (x0y0, x0y1, x1y0, x1y1) of cell base row j.
    p2 = nc.dram_tensor("p2tab", [V, 2 * FEAT], f32)
    with tile.TileContext(nc) as tc:
        with (
            tc.tile_pool(name="build", bufs=2) as bpool,
            tc.tile_pool(name="g", bufs=2) as gpool,
            tc.tile_pool(name="small", bufs=2) as spool,
        ):
            # Interleave on-chip: load rows [r0, r0+R) and [r0+128, r0+128+R)
            # into two tiles (pairs line up per partition), DVE-interleave into
            # [row, pair, 18], store contiguous. All DMAs are big & contiguous.
            ROWS = 8192
            RPP = ROWS // P
            n_bchunks = V // ROWS
            for ci in range(n_bchunks):
                r0 = ci * ROWS
                At = bpool.tile([P, RPP * FEAT], f32, tag="BA")
                Bt = bpool.tile([P, RPP * FEAT], f32, tag="BB")
                nc.scalar.dma_start(
                    out=At[:],
                    in_=cb[r0 : r0 + ROWS, :].rearrange("(p r) e -> p (r e)", p=P))
                if ci < n_bchunks - 1:
                    nc.scalar.dma_start(
                        out=Bt[:],
                        in_=cb[r0 + P : r0 + P + ROWS, :].rearrange("(p r) e -> p (r e)", p=P))
                else:
                    # last 128 pair rows are out of range (y=127 entries,
                    # never indexed) -> zero-fill
                    nc.vector.memset(Bt[:], 0.0)
                    nc.scalar.dma_start(
                        out=Bt[:126, :],
                        in_=cb[r0 + P : V, :].rearrange("(p r) e -> p (r e)", p=126))
                Ot = bpool.tile([P, RPP, 2, FEAT], f32, tag="BO")
                nc.vector.tensor_copy(out=Ot[:, :, 0, :],
                                      in_=At[:].rearrange("p (r e) -> p r e", e=FEAT))
                nc.vector.tensor_copy(out=Ot[:, :, 1, :],
                                      in_=Bt[:].rearrange("p (r e) -> p r e", e=FEAT))
                nc.sync.dma_start(
                    out=p2[r0 : r0 + ROWS, :].rearrange("(p r) e -> p (r e)", p=P),
                    in_=Ot[:].rearrange("p r t e -> p (r t e)"))
            for c in range(n_chunks):
                c0 = c * chunk
                PT = spool.tile([P, 3 * F], f32, tag="PT")
                nc.sync.dma_start(
                    out=PT[:],
                    in_=pts[c0 : c0 + chunk, :].rearrange("(p f) c -> p (f c)", p=P),
                )
                PT3 = PT[:].rearrange("p (f c) -> p f c", c=3)

                Q = spool.tile([P, 3, F], f32, tag="Q")
                FL = spool.tile([P, 3, F], f32, tag="FL")
                W = spool.tile([P, 3, F], f32, tag="W")
                U = spool.tile([P, 3, F], f32, tag="U")
                T = spool.tile([P, 3, F], f32, tag="T")
                # q_k = A[k,0]x + A[k,1]y + A[k,2]z + b_k
                for k in range(3):
                    nc.scalar.activation(Q[:, k, :], PT3[:, :, 0], Copy,
                                         bias=float(b[k]), scale=float(A[k][0]))
                    nc.scalar.activation(T[:, k, :], PT3[:, :, 1], Copy,
                                         bias=0.0, scale=float(A[k][1]))
                    nc.vector.tensor_tensor(out=Q[:, k, :], in0=Q[:, k, :], in1=T[:, k, :], op=Op.add)
                    nc.scalar.activation(T[:, k, :], PT3[:, :, 2], Copy,
                                         bias=0.0, scale=float(A[k][2]))
                    nc.vector.tensor_tensor(out=Q[:, k, :], in0=Q[:, k, :], in1=T[:, k, :], op=Op.add)
                # floor(q): round-to-nearest via magic constant, then fix up
                nc.scalar.activation(T[:], Q[:], Copy, bias=MAGIC)
                nc.scalar.activation(FL[:], T[:], Copy, bias=-MAGIC)
                nc.vector.tensor_tensor(out=T[:], in0=FL[:], in1=Q[:], op=Op.is_gt)
                nc.vector.tensor_tensor(out=FL[:], in0=FL[:], in1=T[:], op=Op.subtract)
                # frac weights (from unclipped floor), then clip floor to [0,126]
                nc.vector.tensor_tensor(out=W[:], in0=Q[:], in1=FL[:], op=Op.subtract)
                nc.vector.tensor_scalar(out=FL[:], in0=FL[:], scalar1=0.0, scalar2=float(RES - 2),
                                        op0=Op.max, op1=Op.min)
                nc.scalar.activation(U[:], W[:], Copy, bias=1.0, scale=-1.0)

                # xy corner weights (dx major, dy minor), then scale by z
                W4 = spool.tile([P, 4, F], f32, tag="W4")
                nc.vector.tensor_tensor(out=W4[:, 0, :], in0=U[:, 0, :], in1=U[:, 1, :], op=Op.mult)
                nc.vector.tensor_tensor(out=W4[:, 1, :], in0=U[:, 0, :], in1=W[:, 1, :], op=Op.mult)
                nc.vector.tensor_tensor(out=W4[:, 2, :], in0=W[:, 0, :], in1=U[:, 1, :], op=Op.mult)
                nc.vector.tensor_tensor(out=W4[:, 3, :], in0=W[:, 0, :], in1=W[:, 1, :], op=Op.mult)
                W8 = spool.tile([P, F, 8], f32, tag="W8")
                for k in range(4):
                    nc.vector.tensor_tensor(out=W8[:, :, k], in0=W4[:, k, :], in1=U[:, 2, :], op=Op.mult)
                    nc.vector.tensor_tensor(out=W8[:, :, 4 + k], in0=W4[:, k, :], in1=W[:, 2, :], op=Op.mult)

                # base row index = fx + 128 fy + 16384 fz  (exact in f32)
                B = spool.tile([P, F], f32, tag="B")
                T2 = spool.tile([P, 2, F], f32, tag="T2")
                nc.scalar.activation(T2[:, 0, :], FL[:, 1, :], Copy, scale=float(RES))
                nc.scalar.activation(T2[:, 1, :], FL[:, 2, :], Copy, scale=float(RES * RES))
                nc.vector.tensor_tensor(out=B[:], in0=FL[:, 0, :], in1=T2[:, 0, :], op=Op.add)
                nc.vector.tensor_tensor(out=B[:], in0=B[:], in1=T2[:, 1, :], op=Op.add)
                IDX = spool.tile([P, F, 2], i32, tag="IDX")
                nc.vector.tensor_copy(out=IDX[:, :, 0], in_=B[:])
                nc.vector.tensor_scalar(out=IDX[:, :, 1], in0=B[:], scalar1=float(RES * RES),
                                        scalar2=None, op0=Op.add)

                # gather: per point-slot f, per z-plane: 72 floats = 4 xy corners
                G = gpool.tile([P, F, 2, 72], f32, tag="G")
                for g in range(F):
                    for zz in range(2):
                        nc.gpsimd.indirect_dma_start(
                            out=G[:, g, zz, :],
                            out_offset=None,
                            in_=p2[:],
                            in_offset=bass.IndirectOffsetOnAxis(ap=IDX[:, g, zz : zz + 1], axis=0),
                        )

                # weighted multiply + in-place tree reduction
                Gv = G[:].rearrange("p f z e -> p (f z e)").rearrange(
                    "p (f d j) -> p f d j", d=8, j=FEAT)
                W8b = W8[:].unsqueeze(-1).broadcast_to([P, F, 8, FEAT])
                nc.vector.tensor_tensor(out=Gv, in0=Gv, in1=W8b, op=Op.mult)
                Gf = G[:].rearrange("p f z e -> p (f z e)")
                for width in (72, 36, 18):
                    a = Gf.rearrange("p (f e) -> p f e", e=144)[:, :, 0:width]
                    bb = Gf.rearrange("p (f e) -> p f e", e=144)[:, :, width : 2 * width]
                    nc.vector.tensor_tensor(out=a, in0=a, in1=bb, op=Op.add)

                res = Gf.rearrange("p (f e) -> p f e", e=144)[:, :, 0:FEAT]
                nc.sync.dma_start(
                    out=out[c0 : c0 + chunk, :].rearrange("(p f) c -> p (f c)", p=P),
                    in_=res,
                )
    nc.finalize()
    return nc


def kernel(pts, codebook, transform, _trace=False):
    from concourse.bass_utils import run_bass_kernel_spmd

    pts = np.asarray(pts, dtype=np.float32)
    codebook = np.ascontiguousarray(np.asarray(codebook, dtype=np.float32))
    transform = np.asarray(transform, dtype=np.float32)

    p_flat = np.ascontiguousarray(pts.reshape(-1, 3))
    n_total = p_flat.shape[0]
    n_per = n_total // N_CORES
    assert n_per * N_CORES == n_total

    # fold transform inverse + grid scale into affine q = A p + b (host side,
    # 4x4 input only)
    R_inv = np.linalg.inv(transform[:3, :3].astype(np.float64))
    A = (RES - 1) * R_inv
    b = -A @ transform[:3, 3].astype(np.float64)

    key = (n_per, A.tobytes(), b.tobytes())
    if key not in _cache:
        _cache[key] = _build(n_per, A, b)
    nc = _cache[key]

    in_maps = [
        {"pts": p_flat[i * n_per : (i + 1) * n_per], "codebook": codebook}
        for i in range(N_CORES)
    ]
    r = run_bass_kernel_spmd(nc, in_maps, list(range(N_CORES)), trace=_trace)
    kernel.last_exec_time_ns = r.exec_time_ns
    out = np.concatenate([r.results[i]["out"] for i in range(N_CORES)], axis=0)
    return out


kernel.last_exec_time_ns = None
